# revision 49
# baseline (speedup 1.0000x reference)
"""DiscRNNG forward pass on 8 Trainium2 NeuronCores (Bass/Tile).

Strategy (batch=1, strictly sequential recurrence):
  - Three independent single-layer LSTM chains (stack, buffer, history) are
    model-parallel: one chain per NeuronCore (cores 3-7 run redundant
    replicas so the SPMD program is uniform).
  - Per-launch input upload is the dominant cost in this environment
    (~5 GB/s for per-core inputs, ~8x cheaper for replicated ones), so
    inputs are aggressively compressed: embeddings / weights quantized to
    fp8-e3m4 with runtime power-of-two scales folded into activation
    `scale` operands, and everything identical-across-cores is passed
    replicated (PartitionSpec()) - including the weights of all 3 cells,
    stacked; each core picks its cell with 3 DVE ops driven by a tiny
    per-core one-hot (exact in fp8 since the mask is 0/1).
  - Per core: embedding projections + x@wih^T contributions for all T steps
    are precomputed as dense matmuls into an SBUF-resident fp8 buffer (no
    DRAM round-trip), then the T=4096 sequential steps run with only the
    h@whh^T matvec + LSTM pointwise ops on the critical path. whh is fp8 so
    LDWEIGHTS runs at FWL rate; gates are host-permuted to (f,i,g,o) so the
    post-matmul pointwise tail is short. Gate psum x-contribution preloads
    are done 32 steps at a time with a single DVE copy into a PSUM bank; PE
    matmuls accumulate onto it (start=False).
  - The per-step h history (fp16 in SBUF for full recurrence precision) is
    exported per 32-step block as fp8-e4m3 (x128) and exchanged with a
    single in-kernel AllToAll so every core computes the softmax head for
    its own T/8 shard. One SPMD launch total.
Embedding gather (4096 rows of the 100k x 300 table) is done host-side.
"""

import os
import sys

sys.path.insert(0, "/opt/trn_rl_repo")

import numpy as np
import ml_dtypes

import concourse.bass as bass
import concourse.mybir as mybir
import concourse.tile as tile
import bass_rust

F8 = mybir.dt.float8e3
F8E4 = mybir.dt.float8e4
F16 = mybir.dt.float16
F32 = mybir.dt.float32
AF = mybir.ActivationFunctionType
MUL = mybir.AluOpType.mult
ADD = mybir.AluOpType.add
E3M4 = ml_dtypes.float8_e3m4
E4M3 = ml_dtypes.float8_e4m3

T, H, G, E, NA = 4096, 512, 2048, 512, 100
X2E = 512            # effective x2 width per cell (w-cells use w2e rows,
                     # the a-cell uses a2e rows)
U = 32
KC = H // 128        # 4
MC = G // 128        # 16
EC = E // 128        # 4
XC2 = X2E // 128     # 4
TCH = 512            # dense precompute time chunk
NCORES = 8
TS = T // NCORES     # 512, head time shard per core
KC2 = 3 * H // 128   # 12, head contraction tiles
DC = H // 128        # 4
TC = TS // 128       # 4
SH = 128.0           # hist fp8 export scale (|h| < 1 always)


def _split_excess_waits(nc, maxw=1):
    """walrus here allows only 1 sync-wait per instruction; hoist excess
    waits onto preceding same-engine nops."""
    for bb in nc.m.functions[0].blocks:
        insts = list(bb.instructions)
        out = []
        changed = False
        for inst in insts:
            si = inst.sync_info
            if si is not None and si.on_wait is not None and len(si.on_wait) > maxw:
                waits = list(si.on_wait)
                keep = waits[-maxw:]
                excess = waits[:-maxw]
                for i in range(0, len(excess), maxw):
                    chunk = excess[i : i + maxw]
                    nop = nc.engines[inst.engine].nop(hint="waitsplit", nofuse=True).ins
                    cur = nc.cur_bb.bb
                    lst = list(cur.instructions)
                    assert lst and lst[-1].name == nop.name
                    cur.instructions = lst[:-1]
                    nop.sync_info = bass_rust.SyncInfo(
                        on_wait=list(chunk), on_update=[]
                    )
                    out.append(nop)
                si.on_wait = keep
                inst.sync_info = si
                changed = True
            out.append(inst)
        if changed:
            bb.instructions = out


# inputs identical on every core - passed replicated (one upload, not 8)
REPL = {"ecatT", "sum_wT", "out_wT", "whh3", "wih3", "wproj3", "smalls"}


def _build_fused(t_loop=T, pointwise=True, head=True, dense=True):
    nc = bass.Bass("TRN2", target_bir_lowering=False, debug=False, num_devices=NCORES)

    # replicated
    ecatT = nc.dram_tensor("ecatT", [E, T], F8, kind="ExternalInput").ap()
    sum_wT = nc.dram_tensor("sum_wT", [3 * H, H], F8, kind="ExternalInput").ap()
    out_wT = nc.dram_tensor("out_wT", [H, NA], F16, kind="ExternalInput").ap()
    # per-cell weight stacks, replicated
    whh3 = nc.dram_tensor("whh3", [3, H, G], F8, kind="ExternalInput").ap()
    wih3 = nc.dram_tensor("wih3", [3, X2E, G], F8, kind="ExternalInput").ap()
    wproj3 = nc.dram_tensor("wproj3", [3, E, X2E], F16, kind="ExternalInput").ap()
    # all small f32 operands packed [128, 172] p-major on host:
    # cols 0:100 out_b, 100:108 scales (invs[3], xsc[3], esc, hsc),
    # 108:112 sum_b, 112:160 bias2[3], 160:172 bproj[3]
    smalls = nc.dram_tensor("smalls", [128, 172], F32, kind="ExternalInput").ap()
    # per-core: cols 0:3 one-hot cell select, 3:7 h0, 7:11 c0
    percore = nc.dram_tensor("percore", [128, 11], F32, kind="ExternalInput").ap()

    outd = nc.dram_tensor("logp", [TS, NA], F32, kind="ExternalOutput").ap()

    from contextlib import ExitStack

    with tile.TileContext(nc) as tc:
        with (
            tc.tile_pool(name="wts", bufs=1) as wts,
            tc.tile_pool(name="ps", bufs=2, space="PSUM") as psp,
            tc.tile_pool(name="state", bufs=1) as statep,
            tc.tile_pool(name="histb", bufs=1) as histbp,
            tc.tile_pool(name="gps", bufs=1, space="PSUM") as gpsp,
            tc.tile_pool(name="p2", bufs=2, space="PSUM") as p2p,
            tc.tile_pool(name="ew", bufs=4) as ewp,
            tc.tile_pool(name="dram", bufs=1, space="DRAM") as dramp,
        ):
            ecat_sb = wts.tile([128, EC * T], F8)
            nc.sync.dma_start(
                ecat_sb[:].rearrange("p (kx t) -> p kx t", kx=EC),
                ecatT.rearrange("(kx p) t -> p kx t", p=128),
            )
            # selected cell weights (targets of the one-hot combine)
            wproj_sb = wts.tile([128, EC * X2E], F16)
            bproj_sb = wts.tile([128, XC2], F32)
            wih2_sb = wts.tile([128, XC2 * G], F8)
            bias2_sb = wts.tile([128, MC], F32)
            whh8_sb = wts.tile([128, KC * G], F8)
            sc_sb = wts.tile([128, 4], F32)  # invs, xsc, esc, hsc
            sm_sb = wts.tile([128, 172], F32)
            nc.sync.dma_start(sm_sb[:], smalls)
            pc_sb = wts.tile([128, 11], F32)
            nc.sync.dma_start(pc_sb[:], percore)
            selv_sb = pc_sb  # cols 0:3
            scl3_sb = sm_sb[:, 100:108]
            ob_sb = sm_sb[:, 0:100]
            sb_sb = sm_sb[:, 108:112]

            with tc.tile_pool(name="selp", bufs=1) as selp:
                whh3_sb = selp.tile([128, 3 * KC * G], F8)
                nc.sync.dma_start(
                    whh3_sb[:].rearrange("p (q kc m) -> p q kc m", q=3, kc=KC),
                    whh3.rearrange("q (kc p) m -> p q kc m", p=128),
                )
                wih3_sb = selp.tile([128, 3 * XC2 * G], F8)
                nc.sync.dma_start(
                    wih3_sb[:].rearrange("p (q kx m) -> p q kx m", q=3, kx=XC2),
                    wih3.rearrange("q (kx p) m -> p q kx m", p=128),
                )
                wproj3_sb = selp.tile([128, 3 * EC * X2E], F16)
                nc.sync.dma_start(
                    wproj3_sb[:].rearrange("p (q kx m) -> p q kx m", q=3, kx=EC),
                    wproj3.rearrange("q (kx p) m -> p q kx m", p=128),
                )
                b2_3_sb = sm_sb[:, 112:160]
                bp3_sb = sm_sb[:, 160:172]

                def sel3(dst, src_sb, width, tmp_tag, dtype):
                    sv = lambda q: selv_sb[:, q : q + 1]
                    t1_ = selp.tile([128, width], dtype, tag=tmp_tag)
                    nc.vector.tensor_scalar_mul(
                        t1_[:], src_sb[:, 0:width], sv(0)
                    )
                    t2_ = selp.tile([128, width], dtype, tag=tmp_tag + "b")
                    nc.vector.scalar_tensor_tensor(
                        t2_[:], src_sb[:, width : 2 * width], sv(1), t1_[:],
                        MUL, ADD,
                    )
                    nc.vector.scalar_tensor_tensor(
                        dst, src_sb[:, 2 * width : 3 * width], sv(2), t2_[:],
                        MUL, ADD,
                    )

                sel3(whh8_sb[:], whh3_sb, KC * G, "twhh", F8)
                sel3(wih2_sb[:], wih3_sb, XC2 * G, "twih", F8)
                sel3(wproj_sb[:], wproj3_sb, EC * X2E, "twp", F16)
                sel3(bias2_sb[:], b2_3_sb, MC, "tb2", F32)
                sel3(bproj_sb[:], bp3_sb, XC2, "tbp", F32)
                sel3(sc_sb[:, 0:1], scl3_sb, 1, "ts1", F32)      # invs
                sel3(sc_sb[:, 1:2], scl3_sb[:, 3:], 1, "ts2", F32)  # xsc
                nc.vector.tensor_copy(sc_sb[:, 2:4], scl3_sb[:, 6:8])

            # head weights
            sw_sb = wts.tile([128, KC2 * H], F8)
            nc.sync.dma_start(
                sw_sb[:].rearrange("p (k m) -> p k m", k=KC2),
                sum_wT.rearrange("(k p) m -> p k m", p=128),
            )
            ow_sb = wts.tile([128, DC * NA], F16)
            nc.sync.dma_start(
                ow_sb[:].rearrange("p (c a) -> p c a", c=DC),
                out_wT.rearrange("(c p) a -> p c a", p=128),
            )

            invs_ap = sc_sb[:, 0:1]
            xsc_ap = sc_sb[:, 1:2]
            esc_ap = sc_sb[:, 2:3]
            hsc_ap = sc_sb[:, 3:4]

            # big SBUF-resident intermediates, opened after selp released its
            # zone: xct (x-contributions for all T, f16) and hist (f8 x128)
            _st = ExitStack()
            bigp = _st.enter_context(tc.tile_pool(name="big", bufs=1))
            x2p = _st.enter_context(tc.tile_pool(name="x2p", bufs=2))
            TP = T + 2 * U
            xct_sb = bigp.tile([128, MC * TP], F8E4)
            # t-major gate interleave (col = t*16 + m): the per-half PSUM
            # preload is then a fully contiguous copy
            xv = xct_sb[:].rearrange("p (t m) -> p t m", m=MC)
            if not dense:
                nc.vector.memset(xct_sb[:, : MC * TP // 2], 0.0)
                nc.vector.memset(xct_sb[:, MC * TP // 2 :], 0.0)
            hist_sb = bigp.tile([128, KC * T], F8E4)
            hv = hist_sb[:].rearrange("p (k t) -> p k t", t=T)

            # precompute XCT = S*(WIH @ relu(Wproj @ ecatT + bproj) + bias2)
            for tc_i in range(T // TCH if dense else 0):
                x2_sb = x2p.tile([128, XC2 * TCH], F16)
                for mx in range(XC2):
                    ps = psp.tile([128, TCH], F32)
                    for kx in range(EC):
                        nc.tensor.matmul(
                            ps[:],
                            wproj_sb[
                                :, kx * X2E + mx * 128 : kx * X2E + (mx + 1) * 128
                            ],
                            ecat_sb[:, kx * T + tc_i * TCH : kx * T + (tc_i + 1) * TCH],
                            start=(kx == 0),
                            stop=(kx == EC - 1),
                        )
                    nc.scalar.activation(
                        x2_sb[:, mx * TCH : (mx + 1) * TCH],
                        ps[:],
                        AF.Relu,
                        bias=bproj_sb[:, mx : mx + 1],
                        scale=esc_ap,
                    )
                for m in range(MC):
                    ps = psp.tile([128, TCH], F32)
                    for kx in range(XC2):
                        nc.tensor.matmul(
                            ps[:],
                            wih2_sb[:, kx * G + m * 128 : kx * G + (m + 1) * 128],
                            x2_sb[:, kx * TCH : (kx + 1) * TCH],
                            start=(kx == 0),
                            stop=(kx == XC2 - 1),
                        )
                    nc.scalar.activation(
                        xv[:, tc_i * TCH : (tc_i + 1) * TCH, m],
                        ps[:], AF.Identity, bias=bias2_sb[:, m : m + 1],
                        scale=xsc_ap,
                    )

            # sequential recurrence, software-pipelined XC prefetch
            h_cur = statep.tile([128, KC], F16)
            c_sb = statep.tile([128, KC], F32)
            nc.vector.tensor_copy(h_cur[:], pc_sb[:, 3 : 3 + KC])
            nc.vector.tensor_copy(c_sb[:], pc_sb[:, 3 + KC : 3 + 2 * KC])

            # persistent gate-psum banks (one per half); preloaded with the
            # 32 steps' x-contributions (gate-interleaved layout: col u*16+m)
            # straight from the SBUF-resident xct, PE accumulates on top with
            # start=False. The preload for each bank is emitted mid-way
            # through the *other* half's step loop.
            gpsA = gpsp.tile([128, U * 16], F32, tag="psA")
            gpsB = gpsp.tile([128, U * 16], F32, tag="psB")

            def preload(psb, xc_ap):
                # xc_ap: [128, U, MC] t-major view into xct_sb (contiguous)
                nc.vector.tensor_copy(
                    psb[:].rearrange("p (u m) -> p u m", m=16), xc_ap
                )

            preload(gpsA, xv[:, 0:U, :])

            def half(xc_sb, psb, hist_ap, tag, pre_next):
                hist_t = histbp.tile([128, KC * U], F16, tag="h" + tag)
                hist_r = hist_t[:].rearrange("p (k u) -> p u k", k=KC)
                nc.vector.tensor_copy(hist_r[:, 0, :], h_cur[:])
                for u in range(U):
                    if u == 8:
                        pre_next()
                    base = u * 16
                    for grp in range(4):  # f, i, g, o
                        for m in range(grp * 4, grp * 4 + 4):
                            for kc in range(KC):
                                nc.tensor.matmul(
                                    psb[:, base + m : base + m + 1],
                                    whh8_sb[
                                        :, kc * G + m * 128 : kc * G + (m + 1) * 128
                                    ],
                                    hist_t[:, kc * U + u : kc * U + u + 1],
                                    start=False,
                                    stop=(kc == KC - 1),
                                )
                        if not pointwise:
                            if grp == 3:
                                so = ewp.tile([128, 4], F32, tag="so")
                                nc.scalar.activation(
                                    so[:], psb[:, base + 12 : base + 16],
                                    AF.Sigmoid, scale=invs_ap,
                                )
                            continue
                        if grp == 0:
                            pass  # f+i sigmoids merged after the i-group
                        elif grp == 1:
                            sfi = ewp.tile([128, 8], F32, tag="sfi")
                            nc.scalar.activation(
                                sfi[:], psb[:, base : base + 8], AF.Sigmoid,
                                scale=invs_ap,
                            )
                            t2 = ewp.tile([128, 4], F32, tag="t2")
                            nc.vector.tensor_mul(t2[:], sfi[:, 0:4], c_sb[:])
                        elif grp == 2:
                            tg = ewp.tile([128, 4], F32, tag="tg")
                            nc.scalar.activation(
                                tg[:], psb[:, base + 8 : base + 12], AF.Tanh,
                                scale=invs_ap,
                            )
                            t1 = ewp.tile([128, 4], F32, tag="t1")
                            nc.vector.tensor_mul(t1[:], sfi[:, 4:8], tg[:])
                            nc.vector.tensor_add(c_sb[:], t1[:], t2[:])
                            tc2 = ewp.tile([128, 4], F32, tag="tc2")
                            nc.scalar.activation(tc2[:], c_sb[:], AF.Tanh)
                        else:
                            so = ewp.tile([128, 4], F32, tag="so")
                            nc.scalar.activation(
                                so[:], psb[:, base + 12 : base + 16], AF.Sigmoid,
                                scale=invs_ap,
                            )
                            if u < U - 1:
                                nc.vector.tensor_mul(
                                    hist_r[:, u + 1, :], so[:], tc2[:]
                                )
                            else:
                                nc.vector.tensor_mul(h_cur[:], so[:], tc2[:])
                # fp8 export of this block's history into SBUF-resident hist
                # (x128, |h|<1 so safe)
                nc.scalar.activation(
                    hist_ap,
                    hist_t[:].rearrange("p (k u) -> p k u", k=KC),
                    AF.Identity, scale=SH,
                )

            with tc.For_i(0, t_loop, 2 * U, hint_engines=(mybir.EngineType.PE,)) as iv:
                half(
                    None, gpsA, hv[:, :, bass.ds(iv, U)], "A",
                    lambda: preload(gpsB, xv[:, U:, :][:, bass.ds(iv, U), :]),
                )
                half(
                    None, gpsB, hv[:, :, U:][:, :, bass.ds(iv, U)], "B",
                    lambda: preload(gpsA, xv[:, 2 * U :, :][:, bass.ds(iv, U), :]),
                )

            if not head:
                z = ewp.tile([128, NA], F32, tag="hz")
                nc.vector.memset(z[:], 0.0)
                for tcc in range(TC):
                    nc.sync.dma_start(outd[tcc * 128 : (tcc + 1) * 128, :], z[:])
            if head:
                # re-layout history into AllToAll shard-major order and
                # exchange: after A2A, out[r] on core c holds rank r's
                # history columns for time shard c. Ranks 0-2 are chains
                # stk/buf/hist.
                a2a_in = dramp.tile([NCORES, KC, 128, TS], F8E4)
                a2a_out = dramp.tile([NCORES, KC, 128, TS], F8E4)
                for s in range(NCORES):
                    nc.sync.dma_start(
                        a2a_in[s].rearrange("k p t -> p k t"),
                        hv[:, :, s * TS : (s + 1) * TS],
                    )
                nc.gpsimd.collective_compute(
                    "AllToAll",
                    mybir.AluOpType.bypass,
                    replica_groups=[list(range(NCORES))],
                    ins=[a2a_in.opt()],
                    outs=[a2a_out.opt()],
                )

                # softmax head on this core's T/8 shard
                top_sb = wts.tile([128, KC2 * TS], F8E4)
                top_r = top_sb[:].rearrange("p (k t) -> p k t", k=KC2)
                for q in range(3):
                    nc.sync.dma_start(
                        top_r[:, q * KC : (q + 1) * KC, :],
                        a2a_out[q].rearrange("k p t -> p k t"),
                    )
                st_sb = wts.tile([128, DC * TS], F16)
                for dc in range(DC):
                    ps = psp.tile([128, TS], F32)
                    for kc in range(KC2):
                        nc.tensor.matmul(
                            ps[:],
                            sw_sb[:, kc * H + dc * 128 : kc * H + (dc + 1) * 128],
                            top_sb[:, kc * TS : (kc + 1) * TS],
                            start=(kc == 0),
                            stop=(kc == KC2 - 1),
                        )
                    nc.scalar.activation(
                        st_sb[:, dc * TS : (dc + 1) * TS],
                        ps[:],
                        AF.Tanh,
                        bias=sb_sb[:, dc : dc + 1],
                        scale=hsc_ap,
                    )
                for tcc in range(TC):
                    ps2 = p2p.tile([128, NA], F32)
                    for dc in range(DC):
                        nc.tensor.matmul(
                            ps2[:],
                            st_sb[
                                :, dc * TS + tcc * 128 : dc * TS + tcc * 128 + 128
                            ],
                            ow_sb[:, dc * NA : (dc + 1) * NA],
                            start=(dc == 0),
                            stop=(dc == DC - 1),
                        )
                    L = ewp.tile([128, NA], F32, tag="hL")
                    nc.vector.tensor_add(L[:], ps2[:], ob_sb)
                    mx = ewp.tile([128, 1], F32, tag="hmx")
                    nc.vector.reduce_max(mx[:], L[:], axis=mybir.AxisListType.X)
                    D = ewp.tile([128, NA], F32, tag="hD")
                    nc.vector.tensor_scalar(
                        D[:], L[:], mx[:], None, mybir.AluOpType.subtract
                    )
                    Ex = ewp.tile([128, NA], F32, tag="hE")
                    nc.scalar.activation(Ex[:], D[:], AF.Exp)
                    s = ewp.tile([128, 1], F32, tag="hs")
                    nc.vector.reduce_sum(s[:], Ex[:], axis=mybir.AxisListType.X)
                    ls = ewp.tile([128, 1], F32, tag="hls")
                    nc.scalar.activation(ls[:], s[:], AF.Ln)
                    O = ewp.tile([128, NA], F32, tag="hO")
                    nc.vector.tensor_scalar(
                        O[:], D[:], ls[:], None, mybir.AluOpType.subtract
                    )
                    nc.sync.dma_start(outd[tcc * 128 : (tcc + 1) * 128, :], O[:])
            _st.close()

    _split_excess_waits(nc)
    return nc


def _make_runner(nc, n_cores=NCORES, repl_names=frozenset()):
    import jax
    from jax.sharding import Mesh, PartitionSpec
    from jax.experimental.shard_map import shard_map
    from concourse import bass2jax
    from concourse.bass2jax import _bass_exec_p, partition_id_tensor

    bass2jax.install_neuronx_cc_hook()

    partition_name = nc.partition_id_tensor.name if nc.partition_id_tensor else None
    in_names, out_names, out_avals, zero_outs = [], [], [], []
    for alloc in nc.m.functions[0].allocations:
        if not isinstance(alloc, mybir.MemoryLocationSet):
            continue
        name = alloc.memorylocations[0].name
        if alloc.kind == "ExternalInput":
            if name != partition_name:
                in_names.append(name)
        elif alloc.kind == "ExternalOutput":
            shape = tuple(alloc.tensor_shape)
            dtype = mybir.dt.np(alloc.dtype)
            out_names.append(name)
            out_avals.append(jax.core.ShapedArray(shape, dtype))
            zero_outs.append(np.zeros(shape, dtype))
    n_params = len(in_names)
    all_in = list(in_names) + list(out_names) + (
        [partition_name] if partition_name else []
    )

    def _body(*args):
        operands = list(args)
        if partition_name:
            operands.append(partition_id_tensor())
        return tuple(
            _bass_exec_p.bind(
                *operands,
                out_avals=tuple(out_avals),
                in_names=tuple(all_in),
                out_names=tuple(out_names),
                lowering_input_output_aliases=(),
                sim_require_finite=True,
                sim_require_nnan=True,
                nc=nc,
            )
        )

    devices = jax.devices()[:n_cores]
    mesh = Mesh(np.asarray(devices), ("core",))
    in_specs = tuple(
        PartitionSpec() if n in repl_names else PartitionSpec("core")
        for n in in_names
    ) + (PartitionSpec("core"),) * len(out_names)
    fn = jax.jit(
        shard_map(
            _body,
            mesh=mesh,
            in_specs=in_specs,
            out_specs=(PartitionSpec("core"),) * len(out_names),
            check_rep=False,
        ),
        keep_unused=True,
    )

    def run(in_maps):
        import jax

        concat_in = []
        for n in in_names:
            if n in repl_names:
                concat_in.append(np.asarray(in_maps[0][n]))
            else:
                concat_in.append(
                    np.concatenate(
                        [np.asarray(in_maps[c][n]) for c in range(n_cores)], axis=0
                    )
                )
        concat_zeros = [
            np.zeros((n_cores * z.shape[0], *z.shape[1:]), z.dtype)
            for z in zero_outs
        ]
        out = fn(*(concat_in + concat_zeros))
        jax.block_until_ready(out)
        return [
            {
                name: np.asarray(out[i]).reshape(n_cores, *out_avals[i].shape)[c]
                for i, name in enumerate(out_names)
            }
            for c in range(n_cores)
        ]

    run.fn = fn
    run.spec = (in_names, out_names, out_avals, zero_outs, n_cores, repl_names)
    return run


_CACHE = {}


def _runner():
    if "f" not in _CACHE:
        _CACHE["f"] = _make_runner(_build_fused(), repl_names=REPL)
    return _CACHE["f"]


def _q8(x, target=14.0, clip=15.5):
    """Quantize to fp8-e3m4 with a power-of-two scale; returns (q, S)."""
    m = max(float(np.abs(x).max()), 1e-6)
    S = float(2.0 ** np.floor(np.log2(target / m)))
    return np.clip(x * S, -clip, clip).astype(E3M4), S


# gate-order permutation (i,f,g,o) -> (f,i,g,o), applied to weight rows
_PERM = np.concatenate(
    [np.arange(512, 1024), np.arange(0, 512), np.arange(1024, 1536),
     np.arange(1536, 2048)]
)


def _prep_inputs(inputs):
    words = np.asarray(inputs["words"]).astype(np.int64)
    pos_tags = np.asarray(inputs["pos_tags"]).astype(np.int64)
    actions = np.asarray(inputs["actions"]).astype(np.int64)

    # host-side embedding gather (4096 of 100k rows), zero-padded to 512
    ecat = np.zeros((T, E), np.float32)
    ecat[:, 0:300] = np.asarray(inputs["word_emb"])[words]
    ecat[:, 300:332] = np.asarray(inputs["pos_emb"])[pos_tags]
    ecat[:, 332:396] = np.asarray(inputs["act_emb"])[actions]
    ecat8, S_e = _q8(ecat.T)

    sw8, S_sw = _q8(np.asarray(inputs["sum_w"], np.float32).T)

    whh3 = np.empty((3, H, G), E3M4)
    wih3 = np.empty((3, X2E, G), E3M4)
    wproj3 = np.zeros((3, E, X2E), np.float16)
    smalls = np.zeros((128, 172), np.float32)
    smalls[:, 0:100] = np.asarray(inputs["out_b"], np.float32)[None, :]
    smalls[:, 108:112] = np.asarray(inputs["sum_b"], np.float32).reshape(4, 128).T
    h0s, c0s = [], []
    for q, (pre, kind) in enumerate([("stk", "w"), ("buf", "w"), ("hist", "a")]):
        wih = np.asarray(inputs[f"{pre}_wih"], np.float32)[_PERM]
        whh = np.asarray(inputs[f"{pre}_whh"], np.float32)[_PERM]
        bias = (
            np.asarray(inputs[f"{pre}_bih"], np.float32)
            + np.asarray(inputs[f"{pre}_bhh"], np.float32)
        )[_PERM]
        whh8, S = _q8(whh)
        wih8, S_w2 = _q8(wih)
        whh3[q] = whh8.T
        wih3[q] = wih8.T
        smalls[:, 112 + q * MC : 112 + (q + 1) * MC] = (
            (bias * S).reshape(MC, 128).T
        )
        if kind == "w":
            wproj3[q, 0:332, :] = np.asarray(inputs["w2e_w"]).T
            bp = np.asarray(inputs["w2e_b"], np.float32)
        else:
            wproj3[q, 332:396, :] = np.asarray(inputs["a2e_w"]).T
            bp = np.asarray(inputs["a2e_b"], np.float32)
        smalls[:, 160 + q * XC2 : 160 + (q + 1) * XC2] = (
            bp.reshape(XC2, 128).T
        )
        smalls[:, 100 + q] = 1.0 / S
        smalls[:, 103 + q] = S / S_w2
        h0s.append(np.asarray(inputs[f"{pre}_h0"]).reshape(4, 128).T)
        c0s.append(np.asarray(inputs[f"{pre}_c0"]).reshape(4, 128).T)
    smalls[:, 106] = 1.0 / S_e
    smalls[:, 107] = 1.0 / (S_sw * SH)

    shared_vals = dict(
        ecatT=np.ascontiguousarray(ecat8),
        sum_wT=np.ascontiguousarray(sw8),
        out_wT=np.ascontiguousarray(np.asarray(inputs["out_w"]).T).astype(
            np.float16
        ),
        whh3=whh3,
        wih3=wih3,
        wproj3=wproj3,
        smalls=smalls,
    )
    in_maps = []
    for c in range(NCORES):
        q = c % 3
        pc = np.zeros((128, 11), np.float32)
        pc[:, q] = 1.0
        pc[:, 3:7] = h0s[q]
        pc[:, 7:11] = c0s[q]
        in_maps.append(dict(percore=pc, **shared_vals))
    return in_maps


def kernel(**inputs):
    run = _runner()
    in_maps = _prep_inputs(inputs)
    res = run(in_maps)
    return np.concatenate([res[c]["logp"] for c in range(NCORES)], axis=0).astype(
        np.float32
    )
